# revision 105
# baseline (speedup 1.0000x reference)
"""Distributed Trainium2 kernel for GQA sliding-window attention w/ softcap.

Sharding: 8 cores = fsdp(batch)=2 x tp(heads)=4. Core c handles batch c//4,
q-heads [4r:4r+4], kv-heads [2r:2r+2] (r = c%4). Each core computes its
partial output projection (sum over its 4 heads); host sums the 4 tp partials
per batch (the unshard step).

Engine-balanced, software-pipelined design (~229 us/core on the TRN2 cost
model vs 486 us for the naive phase-serial version; PE ~93% busy at the
bf16 roofline):
- Logits built transposed ([S_block, Tq]) so probs feed PV with no transposes.
- Band blocks are column-trimmed to the valid query range (staircase), so
  QK/PV/tanh/exp only touch live columns (~25% less attention work).
- Triangle masking is folded into the QK PSUM accumulation as an extra
  identity-matmul adding -1e5 to dead entries (tanh -> -1, exp -> ~0), which
  keeps the per-block latency chain PE->Act->PE with no DVE/Pool hop.
- Softmax denominator: e-tiles accumulated into esum (DVE bf16 adds; Pool
  for the back chunks), then one gpsimd partition_all_reduce per
  (chunk,head) yields the partition-replicated sum directly -> recip (DVE,
  bf16) -> enc = pv * recip on DVE (TensorTensor allows one PSUM operand).
- RoPE as 5 ops: 3 DVE muls vs duplicated-row fp16 cos/sin tables (the
  swapped-half products are written half-at-a-time to keep all SBUF operands
  of each op on one start partition, which the BIR verifier requires), plus
  sub/add on Pool (SBUF-only there: Pool cannot touch PSUM).
- Weights/x DMAed in batched, dependency-ordered transfers (SP serializes
  descriptor+transfer per dma_start); x tiles issued from the Pool queue so
  they overlap the SP weight stream.
- Emission order interleaves proj(c+1)/oproj(older) matmul slices between
  attention blocks, paced evenly, so PE never drains while Act grinds
  tanh/exp; oproj PSUM double-buffers across two pools; output partials in
  bf16 summed on host.
"""

import numpy as np
import ml_dtypes

B, T, D, H = 2, 2048, 2048, 128
NQ, NKV = 16, 8
HL, KL = 4, 2          # q heads / kv heads per core
WINDOW = 1024
SOFT_CAP = 50.0
CHUNK = 512
NC_CHUNK = T // CHUNK  # 4
NBLK = T // 128        # 16

LAST_RESULT = None

bf16 = ml_dtypes.bfloat16


def _band(c):
    """Ordered blocks for q-chunk c: list of (j, off, width, tri) with the
    full-width block first. tri: None | ('u', col) upper-incl | ('l', col)
    strict-lower, col = start column of the 128-wide triangle group."""
    # valid j range: max(0, 4c-8) .. 4c+3
    jlo, jhi = max(0, 4 * c - 8), 4 * c + 4
    full, left, right = [], [], []
    for j in range(jlo, jhi):
        d = j - 4 * c
        if -4 <= d <= -1:
            full.append((j, 0, CHUNK, None))
        elif d <= -5:
            w = 128 * (d + 9)
            left.append((j, 0, w, ('l', w - 128)))
        else:  # 0..3
            off = 128 * d
            right.append((j, off, CHUNK - off, ('u', off)))
    if full:
        return full + left + right
    # c == 0: no full blocks; delta 0 is full-width (with triangle mask)
    return right + left


def _build_graph():
    import concourse.bass as bass
    import concourse.mybir as mybir
    from concourse import bacc, bass_isa
    from concourse.tile import TileContext
    from contextlib import ExitStack

    dt = mybir.dt
    AF = mybir.ActivationFunctionType
    nc = bacc.Bacc()

    # weight/x layouts pre-arranged host-side: [128 partitions, 16 D-slices, cols]
    xt = nc.declare_dram_parameter("xt", [128, 16, T], dt.bfloat16, isOutput=False)
    wq = nc.declare_dram_parameter("wq", [HL, 128, 16, H], dt.bfloat16, isOutput=False)
    wk = nc.declare_dram_parameter("wk", [KL, 128, 16, H], dt.bfloat16, isOutput=False)
    wv = nc.declare_dram_parameter("wv", [128, 16, KL * H], dt.bfloat16, isOutput=False)
    wo = nc.declare_dram_parameter("wo", [HL, H, D], dt.bfloat16, isOutput=False)
    rcos = nc.declare_dram_parameter("rcos", [128, T], dt.float16, isOutput=False)
    rsin = nc.declare_dram_parameter("rsin", [128, T], dt.float16, isOutput=False)
    # mconst[0]=identity, [1]=upper-mask bias (-1e5 where s>q), [2]=lower bias
    mconst = nc.declare_dram_parameter("mconst", [3, 128, 128], dt.bfloat16, isOutput=False)
    out = nc.declare_dram_parameter("out", [T, D], dt.bfloat16, isOutput=True)

    with TileContext(nc) as tc, ExitStack() as ctx:
        p_wq = ctx.enter_context(tc.tile_pool(name="wq", bufs=HL))
        p_wk = ctx.enter_context(tc.tile_pool(name="wk", bufs=KL))
        p_wv = ctx.enter_context(tc.tile_pool(name="wv", bufs=1))
        p_wo = ctx.enter_context(tc.tile_pool(name="wo", bufs=HL))
        p_tab = ctx.enter_context(tc.tile_pool(name="tab", bufs=2))
        p_tri = ctx.enter_context(tc.tile_pool(name="tri", bufs=3))
        p_qt = ctx.enter_context(tc.tile_pool(name="qt", bufs=8))
        p_kt = ctx.enter_context(tc.tile_pool(name="kt", bufs=KL * NC_CHUNK))
        p_v = ctx.enter_context(tc.tile_pool(name="v", bufs=NBLK))
        p_xt = ctx.enter_context(tc.tile_pool(name="xt", bufs=3))
        p_rt = ctx.enter_context(tc.tile_pool(name="rt", bufs=8))
        p_e = ctx.enter_context(tc.tile_pool(name="e", bufs=6))
        p_es = ctx.enter_context(tc.tile_pool(name="es", bufs=2))
        p_rc = ctx.enter_context(tc.tile_pool(name="rc", bufs=4))
        p_enc = ctx.enter_context(tc.tile_pool(name="enc", bufs=12))
        p_ost = ctx.enter_context(tc.tile_pool(name="ost", bufs=4))
        ps_lp = ctx.enter_context(tc.tile_pool(name="pslp", bufs=3, space="PSUM"))
        ps_pv = ctx.enter_context(tc.tile_pool(name="pspv", bufs=1, space="PSUM"))
        ps_pj = ctx.enter_context(tc.tile_pool(name="pspj", bufs=2, space="PSUM"))
        ps_op = ctx.enter_context(tc.tile_pool(name="psop", bufs=1, space="PSUM"))
        ps_ms = ctx.enter_context(tc.tile_pool(name="psms", bufs=1, space="PSUM"))

        # --- persistent weight / table loads (batched DMAs, dep-order) ---
        wq_sb = [p_wq.tile([128, 16, H], dt.bfloat16, tag="wq", name="wqt") for _ in range(HL)]
        wk_sb = [p_wk.tile([128, 16, H], dt.bfloat16, tag="wk", name="wkt") for _ in range(KL)]
        wv_sb = p_wv.tile([128, 16, KL * H], dt.bfloat16, tag="wv", name="wvt")
        wo_sb = [p_wo.tile([128, D], dt.bfloat16, tag="wo", name="wot") for _ in range(HL)]
        rcos_sb = p_tab.tile([128, T], dt.float16, tag="tab")
        rsin_sb = p_tab.tile([128, T], dt.float16, tag="tab")
        mc_sb = [p_tri.tile([128, 128], dt.bfloat16, tag="tri", name="trit") for _ in range(3)]

        def dma_xt(c):
            # issued from the Pool queue: runs concurrently with SP's weight DMAs
            cc = slice(c * CHUNK, (c + 1) * CHUNK)
            t = p_xt.tile([128, 16, CHUNK], dt.bfloat16, tag="xt", name="xtt")
            for d0 in range(0, 16, 4):
                nc.gpsimd.dma_start(t[:, d0:d0 + 4, :], xt[:, d0:d0 + 4, cc])
            return t

        # chunk-0 dependencies first: wk0 (quartered so the first proj
        # matmuls start ASAP), rope tables, wk1, wv, then the rest
        for d0 in range(0, 16, 4):
            nc.sync.dma_start(wk_sb[0][:, d0:d0 + 4, :], wk[0, :, d0:d0 + 4, :])
        xts0 = p_xt.tile([128, 16, CHUNK], dt.bfloat16, tag="xt", name="xtt")
        for d0 in range(0, 16, 2):  # eighths: finer-grained arrival
            nc.gpsimd.dma_start(xts0[:, d0:d0 + 2, :], xt[:, d0:d0 + 2, 0:CHUNK])
        nc.sync.dma_start(rcos_sb[:], rcos[:])
        nc.sync.dma_start(rsin_sb[:], rsin[:])
        nc.sync.dma_start(wk_sb[1][:], wk[1])
        nc.sync.dma_start(wv_sb[:], wv[:])
        for h in range(HL):
            nc.sync.dma_start(wq_sb[h][:], wq[h])
        for m in range(3):
            nc.sync.dma_start(mc_sb[m][:], mconst[m, :, :])
        for h in range(HL):
            nc.sync.dma_start(wo_sb[h][:], wo[h, :, :])

        qt_sb = {}   # (h, c) -> tile, chunk-local
        kt_sb = {(h, c): p_kt.tile([128, CHUNK], dt.bfloat16, tag="kt", name="ktt")
                 for h in range(KL) for c in range(NC_CHUNK)}
        v_sb = [p_v.tile([128, KL * H], dt.bfloat16, tag="v", name="vt") for _ in range(NBLK)]
        enc_sb = {}

        def rope(ps, cc, dst):
            # dst[0:64]  = ps[0:64]*cos - ps[64:128]*sin
            # dst[64:128]= ps[64:128]*cos + ps[0:64]*sin
            # rcos/rsin tables carry the 64-row block duplicated to 128 rows.
            # Muls (PSUM reads) on DVE, partition-aligned; sub/add on Pool.
            tc_ = p_rt.tile([128, CHUNK], dt.float32, tag="rt")
            ts_ = p_rt.tile([128, CHUNK], dt.float32, tag="rt")
            yield nc.vector.tensor_mul(tc_[:], ps[:], rcos_sb[:, cc])
            yield nc.vector.tensor_mul(ts_[0:64, :], ps[64:128, :], rsin_sb[0:64, cc])
            yield nc.vector.tensor_mul(ts_[64:128, :], ps[0:64, :], rsin_sb[64:128, cc])
            yield nc.gpsimd.tensor_sub(dst[0:64, :], tc_[0:64, :], ts_[0:64, :])
            yield nc.gpsimd.tensor_add(dst[64:128, :], tc_[64:128, :], ts_[64:128, :])

        def proj_gen(c, xts):
            """q/k/v projections for chunk c; yields between small slices.
            Order: k heads, q head 0 (so the next chunk's first attention
            head can start promptly), v blocks, then q heads 1-3."""
            cc = slice(c * CHUNK, (c + 1) * CHUNK)

            def kproj(h):
                ps = ps_pj.tile([128, CHUNK], dt.float32, tag="pj")
                for d0 in range(0, 16, 4):
                    for d in range(d0, d0 + 4):
                        nc.tensor.matmul(ps[:], wk_sb[h][:, d, :], xts[:, d, :],
                                         start=(d == 0), stop=(d == 15))
                    yield
                for _ in rope(ps, cc, kt_sb[(h, c)]):
                    yield

            def vproj(p):
                ps = ps_pj.tile([128, KL * H], dt.float32, tag="pj")
                for d0 in range(0, 16, 4):
                    for d in range(d0, d0 + 4):
                        nc.tensor.matmul(ps[:], xts[:, d, p * 128:(p + 1) * 128], wv_sb[:, d, :],
                                         start=(d == 0), stop=(d == 15))
                    yield
                nc.vector.tensor_copy(v_sb[c * 4 + p][:], ps[:])
                yield

            def qproj(h):
                ps = ps_pj.tile([128, CHUNK], dt.float32, tag="pj")
                for d0 in range(0, 16, 4):
                    for d in range(d0, d0 + 4):
                        nc.tensor.matmul(ps[:], wq_sb[h][:, d, :], xts[:, d, :],
                                         start=(d == 0), stop=(d == 15))
                    yield
                qt = p_qt.tile([128, CHUNK], dt.bfloat16, tag="qt", name="qtt")
                for _ in rope(ps, cc, qt):
                    yield
                qt_sb[(h, c)] = qt

            units = [kproj(0), kproj(1)] + [vproj(p) for p in range(4)] \
                + [qproj(h) for h in range(HL)]
            for u in units:
                yield from u

        def oproj_gen(c, lo=0, hi=16):
            """output projection tiles [lo,hi) for chunk c; yields per tile.
            PSUM alternates ps_op/ps_ms (double-buffer); PSUM->SBUF copies go
            to Act in phases where it has slack, else DVE."""
            for i in range(lo, hi):
                p, dc = i // 4, i % 4
                tq = c * 4 + p
                # double-buffer PSUM via ps_ms (free of dn tiles now)
                pool = ps_ms if i % 2 else ps_op
                ps = pool.tile([128, 512], dt.float32,
                               tag="ms" if pool is ps_ms else "op")
                for h in range(HL):
                    nc.tensor.matmul(ps[:], enc_sb[(c, h)][:, p * 128:(p + 1) * 128],
                                     wo_sb[h][:, dc * 512:(dc + 1) * 512],
                                     start=(h == 0), stop=(h == HL - 1))
                orow = out[tq * 128:(tq + 1) * 128, dc * 512:(dc + 1) * 512]
                ot = p_ost.tile([128, 512], dt.bfloat16, tag="ost")
                if c == 3:        # tail: Act is idlest there
                    nc.scalar.copy(ot[:], ps[:])
                else:             # keep Act pure tanh/exp during attention
                    nc.vector.tensor_copy(ot[:], ps[:])
                nc.sync.dma_start(orow, ot[:])
                yield

        def attn_head_gen(c, h):
            kv = h // 2
            blocks = _band(c)
            n = len(blocks)
            qt = qt_sb[(h, c)]
            pv = ps_pv.tile([128, CHUNK], dt.float32, tag="pv")
            es = p_es.tile([128, CHUNK], dt.bfloat16, tag="es")
            lps = {}
            es_tiles = {}

            def emit_qk(i):
                j, off, w, trim = blocks[i]
                lp = ps_lp.tile([128, CHUNK], dt.float32, tag="lp")
                lps[i] = lp
                nc.tensor.matmul(lp[:, 0:w], kt_sb[(kv, j // 4)][:, (j % 4) * 128:(j % 4 + 1) * 128],
                                 qt[:, off:off + w], start=True, stop=(trim is None))
                if trim is not None:
                    # fold the triangle mask in as an additive -1e5 bias:
                    # lp[:, tri cols] += I.T @ mask_bias  (53ns PE pass)
                    kind, col = trim
                    msk = mc_sb[1] if kind == 'u' else mc_sb[2]
                    lc = col - off
                    nc.tensor.matmul(lp[:, lc:lc + 128], mc_sb[0][:], msk[:],
                                     start=False, stop=True, skip_group_check=True)

            def finish(i):
                j, off, w, trim = blocks[i]
                lp = lps.pop(i)
                nc.scalar.activation(lp[:, 0:w], lp[:, 0:w], AF.Tanh, scale=1.0 / SOFT_CAP)
                e = p_e.tile([128, CHUNK], dt.bfloat16, tag="e")
                nc.scalar.activation(e[:, 0:w], lp[:, 0:w], AF.Exp, scale=SOFT_CAP)
                # chunk 3 has no proj filler: its esum rides the idle Pool
                eng = nc.gpsimd if c >= 2 else nc.vector
                if i == 0:
                    assert off == 0 and w == CHUNK
                    eng.tensor_copy(es[:], e[:])
                else:
                    eng.tensor_add(es[:, off:off + w], es[:, off:off + w], e[:, 0:w])
                nc.tensor.matmul(pv[:, off:off + w], v_sb[j][:, kv * 128:(kv + 1) * 128],
                                 e[:, 0:w], start=(i == 0), stop=(i == n - 1),
                                 skip_group_check=True)

            LOOKAHEAD = 2
            for i in range(min(LOOKAHEAD, n)):
                emit_qk(i)
            for i in range(n):
                if i + LOOKAHEAD < n:
                    emit_qk(i + LOOKAHEAD)
                finish(i)
                yield
            # head epilogue: denominator, reciprocal, broadcast, normalize
            # denominator + broadcast in one Pool ucode op (output is the
            # partition-replicated sum), freeing PE of the dn matmul and
            # ps_ms of the dn tile
            dnb = p_rc.tile([128, CHUNK], dt.float32, tag="dnb")
            nc.gpsimd.partition_all_reduce(dnb[:], es[:], 128, bass_isa.ReduceOp.add)
            rcb = p_rc.tile([128, CHUNK], dt.bfloat16, tag="bcs")
            with nc.allow_low_precision(reason="bf16 softmax normalizer is ample"):
                nc.vector.reciprocal(rcb[:], dnb[:])
            enc = p_enc.tile([128, CHUNK], dt.bfloat16, tag="enc")
            nc.vector.tensor_mul(enc[:], pv[:], rcb[:])
            enc_sb[(c, h)] = enc
            yield

        def pump_all(gens):
            for g in gens:
                for _ in g:
                    pass

        # --- schedule: flattened stream of 16 (chunk, head) attention units
        # with proj/oproj generators registered as fillers when their deps
        # are met, paced globally so PE stays interleaved end to end ---
        xts1 = dma_xt(1)
        g0 = proj_gen(0, xts0)
        glen = {}           # gen -> remaining yield estimate

        def mk(g, n):
            glen[g] = n
            return g

        # yields per proj_gen (2k*9 + 4v*5 + 4q*9) + 1 so the trailing
        # StopIteration call still runs the generator's tail code (the last
        # q head's qt_sb registration happens after its final yield)
        PROJ_Y = 75
        projg = {0: mk(g0, PROJ_Y), 1: mk(proj_gen(1, xts1), PROJ_Y)}
        filler = [g0, projg[1]]

        def pump_gen(g):
            try:
                next(g)
                glen[g] -= 1
                return True
            except StopIteration:
                glen[g] = 0
                return False

        fi = 0

        def pump(k):
            nonlocal fi
            misses = 0
            while k > 0 and misses < len(filler):
                g = filler[fi % len(filler)]
                fi += 1
                if glen.get(g, 0) > 0 and pump_gen(g):
                    k -= 1
                    misses = 0
                else:
                    misses += 1

        # prologue: emit proj(0) up to the first q head, queue PE lookahead
        while (0, 0) not in qt_sb:
            pump_gen(g0)
        pump(8)

        Y_total = sum((len(_band(c)) + 1) * HL for c in range(NC_CHUNK))
        y_done = 0
        pump_acc = 0.0
        # oproj registration points, tuned so the act-bound attn(3) phase
        # keeps enough PE filler: oproj(0) splits across attn(1)/(2),
        # oproj(1)+(2) land in attn(3), oproj(3) drains in the tail
        defer = {(1, 0): [(oproj_gen(0, 0, 8), 8)],
                 (2, 0): [(oproj_gen(0, 8, 16), 8)],
                 (3, 0): [(oproj_gen(1), 16), (oproj_gen(2), 16)]}
        for c in range(NC_CHUNK):
            for h in range(HL):
                if h == 0 and c + 1 <= 3 and c + 1 not in projg:
                    projg[c + 1] = mk(proj_gen(c + 1, dma_xt(c + 1)), PROJ_Y)
                    filler.append(projg[c + 1])
                for g, n in defer.get((c, h), []):
                    filler.append(mk(g, n))
                gq = projg.get(c)
                while (h, c) not in qt_sb and glen.get(gq, 0) > 0:
                    pump_gen(gq)
                for _ in attn_head_gen(c, h):
                    y_done += 1
                    left = Y_total - y_done
                    rem = sum(glen.get(g2, 0) for g2 in filler)
                    if left > 0:
                        # proportional pacing via fractional accumulator so
                        # filler never runs dry before the stream ends
                        pump_acc += rem / left
                        n = int(pump_acc)
                        if n:
                            pump_acc -= n
                            pump(n)
                    else:
                        pump(rem)
        filler.append(mk(oproj_gen(3), 16))
        for g in filler:
            while glen.get(g, 0) > 0:
                if not pump_gen(g):
                    break

    nc.compile()
    return nc


def _rope_tables(positions):
    frac = 2.0 * np.arange(64) / H
    timescale = 10000.0 ** frac
    ang = positions[None, :].astype(np.float64) / timescale[:, None]
    cos = np.cos(ang).astype(np.float16)
    sin = np.sin(ang).astype(np.float16)
    # rows duplicated so both halves can be handled with [128,*] ops
    return (np.concatenate([cos, cos], axis=0),
            np.concatenate([sin, sin], axis=0))


def _build_mconst():
    sig = np.arange(128)[:, None]
    tau = np.arange(128)[None, :]
    mc = np.zeros((3, 128, 128), dtype=np.float32)
    mc[0] = (sig == tau)                   # identity (mask-add lhsT)
    mc[1] = np.where(sig <= tau, 0.0, -1e5)  # keep upper incl diag
    mc[2] = np.where(sig > tau, 0.0, -1e5)   # keep strict lower
    return mc.astype(bf16)


def _build_in_maps(x, segment_pos, attn_mask, wq, wkv, wo):
    x = np.asarray(x)
    segment_pos = np.asarray(segment_pos)
    wq = np.asarray(wq)
    wkv = np.asarray(wkv)
    wo = np.asarray(wo)
    mc_np = _build_mconst()
    scale = H ** -0.5

    def dslice(a):
        # [D, C] -> [128, 16, C]: partition-major over 128-row D slices
        return np.ascontiguousarray(
            a.reshape(16, 128, a.shape[1]).transpose(1, 0, 2))

    in_maps = []
    for c in range(8):
        b, r = c // 4, c % 4
        cos, sin = _rope_tables(segment_pos[b])
        in_maps.append({
            "xt": dslice(x[b].T).astype(bf16),
            "wq": np.stack([dslice(wq[4 * r + h] * scale) for h in range(4)]).astype(bf16),
            "wk": np.stack([dslice(wkv[0, 2 * r + h]) for h in range(2)]).astype(bf16),
            "wv": dslice(np.concatenate(
                [wkv[1, 2 * r], wkv[1, 2 * r + 1]], axis=1)).astype(bf16),
            "wo": wo[4 * r:4 * r + 4].astype(bf16),
            "rcos": cos, "rsin": sin,
            "mconst": mc_np,
        })
    return in_maps


def kernel(x, segment_pos, attn_mask, wq, wkv, wo):
    global LAST_RESULT
    from concourse.bass_utils import run_bass_kernel_spmd

    nc = _build_graph()
    in_maps = _build_in_maps(x, segment_pos, attn_mask, wq, wkv, wo)

    res = run_bass_kernel_spmd(nc, in_maps, core_ids=list(range(8)))
    LAST_RESULT = res
    out = np.zeros((B, T, D), dtype=np.float32)
    for c in range(8):
        out[c // 4] += res.results[c]["out"].astype(np.float32)
    return out


# revision 111
# speedup vs baseline: 1.0057x; 1.0057x over previous
"""Distributed Trainium2 kernel for GQA sliding-window attention w/ softcap.

Sharding: 8 cores = fsdp(batch)=2 x tp(heads)=4. Core c handles batch c//4,
q-heads [4r:4r+4], kv-heads [2r:2r+2] (r = c%4). Each core computes its
partial output projection (sum over its 4 heads); host sums the 4 tp partials
per batch (the unshard step).

Engine-balanced, software-pipelined design (~228 us/core on the TRN2 cost
model vs 486 us for the naive phase-serial version; PE ~93% busy at the
bf16 roofline):
- Logits built transposed ([S_block, Tq]) so probs feed PV with no transposes.
- Band blocks are column-trimmed to the valid query range (staircase), so
  QK/PV/tanh/exp only touch live columns (~25% less attention work).
- Triangle masking is folded into the QK PSUM accumulation as an extra
  identity-matmul adding -1e5 to dead entries (tanh -> -1, exp -> ~0), which
  keeps the per-block latency chain PE->Act->PE with no DVE/Pool hop.
- Softmax denominator: e-tiles accumulated into esum (DVE bf16 adds; Pool
  for the back chunks), then one gpsimd partition_all_reduce per
  (chunk,head) yields the partition-replicated sum directly -> recip (DVE,
  bf16) -> enc = pv * recip on DVE (TensorTensor allows one PSUM operand).
- RoPE as 5 ops: 3 DVE muls vs duplicated-row fp16 cos/sin tables (the
  swapped-half products are written half-at-a-time to keep all SBUF operands
  of each op on one start partition, which the BIR verifier requires), plus
  sub/add on Pool (SBUF-only there: Pool cannot touch PSUM).
- Weights/x DMAed in batched, dependency-ordered transfers (SP serializes
  descriptor+transfer per dma_start); x tiles issued from the Pool queue so
  they overlap the SP weight stream.
- Emission order interleaves proj(c+1)/oproj(older) matmul slices between
  attention blocks, paced evenly, so PE never drains while Act grinds
  tanh/exp; oproj PSUM double-buffers across two pools; output partials in
  bf16 summed on host.
"""

import numpy as np
import ml_dtypes

B, T, D, H = 2, 2048, 2048, 128
NQ, NKV = 16, 8
HL, KL = 4, 2          # q heads / kv heads per core
WINDOW = 1024
SOFT_CAP = 50.0
CHUNK = 512
NC_CHUNK = T // CHUNK  # 4
NBLK = T // 128        # 16

LAST_RESULT = None

bf16 = ml_dtypes.bfloat16


def _band(c):
    """Ordered blocks for q-chunk c: list of (j, off, width, tri) with the
    full-width block first. tri: None | ('u', col) upper-incl | ('l', col)
    strict-lower, col = start column of the 128-wide triangle group."""
    # valid j range: max(0, 4c-8) .. 4c+3
    jlo, jhi = max(0, 4 * c - 8), 4 * c + 4
    full, left, right = [], [], []
    for j in range(jlo, jhi):
        d = j - 4 * c
        if -4 <= d <= -1:
            full.append((j, 0, CHUNK, None))
        elif d <= -5:
            w = 128 * (d + 9)
            left.append((j, 0, w, ('l', w - 128)))
        else:  # 0..3
            off = 128 * d
            right.append((j, off, CHUNK - off, ('u', off)))
    if full:
        return full + left + right
    # c == 0: no full blocks; delta 0 is full-width (with triangle mask)
    return right + left


def _build_graph():
    import concourse.bass as bass
    import concourse.mybir as mybir
    from concourse import bacc, bass_isa
    from concourse.tile import TileContext
    from contextlib import ExitStack

    dt = mybir.dt
    AF = mybir.ActivationFunctionType
    nc = bacc.Bacc()

    # weight/x layouts pre-arranged host-side: [128 partitions, 16 D-slices, cols]
    xt = nc.declare_dram_parameter("xt", [128, 16, T], dt.bfloat16, isOutput=False)
    wq = nc.declare_dram_parameter("wq", [HL, 128, 16, H], dt.bfloat16, isOutput=False)
    wk = nc.declare_dram_parameter("wk", [KL, 128, 16, H], dt.bfloat16, isOutput=False)
    wv = nc.declare_dram_parameter("wv", [128, 16, KL * H], dt.bfloat16, isOutput=False)
    wo = nc.declare_dram_parameter("wo", [HL, H, D], dt.bfloat16, isOutput=False)
    rcos = nc.declare_dram_parameter("rcos", [128, T], dt.float16, isOutput=False)
    rsin = nc.declare_dram_parameter("rsin", [128, T], dt.float16, isOutput=False)
    # mconst[0]=identity, [1]=upper-mask bias (-1e5 where s>q), [2]=lower bias
    mconst = nc.declare_dram_parameter("mconst", [3, 128, 128], dt.bfloat16, isOutput=False)
    out = nc.declare_dram_parameter("out", [T, D], dt.bfloat16, isOutput=True)

    with TileContext(nc) as tc, ExitStack() as ctx:
        p_wq = ctx.enter_context(tc.tile_pool(name="wq", bufs=HL))
        p_wk = ctx.enter_context(tc.tile_pool(name="wk", bufs=KL))
        p_wv = ctx.enter_context(tc.tile_pool(name="wv", bufs=1))
        p_wo = ctx.enter_context(tc.tile_pool(name="wo", bufs=HL))
        p_tab = ctx.enter_context(tc.tile_pool(name="tab", bufs=2))
        p_tri = ctx.enter_context(tc.tile_pool(name="tri", bufs=3))
        p_qt = ctx.enter_context(tc.tile_pool(name="qt", bufs=8))
        p_kt = ctx.enter_context(tc.tile_pool(name="kt", bufs=KL * NC_CHUNK))
        p_v = ctx.enter_context(tc.tile_pool(name="v", bufs=NBLK))
        p_xt = ctx.enter_context(tc.tile_pool(name="xt", bufs=3))
        p_rt = ctx.enter_context(tc.tile_pool(name="rt", bufs=8))
        p_e = ctx.enter_context(tc.tile_pool(name="e", bufs=6))
        p_es = ctx.enter_context(tc.tile_pool(name="es", bufs=2))
        p_rc = ctx.enter_context(tc.tile_pool(name="rc", bufs=4))
        p_enc = ctx.enter_context(tc.tile_pool(name="enc", bufs=12))
        p_ost = ctx.enter_context(tc.tile_pool(name="ost", bufs=4))
        p_warm = ctx.enter_context(tc.tile_pool(name="warm", bufs=1))
        ps_lp = ctx.enter_context(tc.tile_pool(name="pslp", bufs=3, space="PSUM"))
        ps_pv = ctx.enter_context(tc.tile_pool(name="pspv", bufs=1, space="PSUM"))
        ps_pj = ctx.enter_context(tc.tile_pool(name="pspj", bufs=2, space="PSUM"))
        ps_op = ctx.enter_context(tc.tile_pool(name="psop", bufs=1, space="PSUM"))
        ps_ms = ctx.enter_context(tc.tile_pool(name="psms", bufs=1, space="PSUM"))

        # --- persistent weight / table loads (batched DMAs, dep-order) ---
        wq_sb = [p_wq.tile([128, 16, H], dt.bfloat16, tag="wq", name="wqt") for _ in range(HL)]
        wk_sb = [p_wk.tile([128, 16, H], dt.bfloat16, tag="wk", name="wkt") for _ in range(KL)]
        wv_sb = p_wv.tile([128, 16, KL * H], dt.bfloat16, tag="wv", name="wvt")
        wo_sb = [p_wo.tile([128, D], dt.bfloat16, tag="wo", name="wot") for _ in range(HL)]
        rcos_sb = p_tab.tile([128, T], dt.float16, tag="tab")
        rsin_sb = p_tab.tile([128, T], dt.float16, tag="tab")
        mc_sb = [p_tri.tile([128, 128], dt.bfloat16, tag="tri", name="trit") for _ in range(3)]

        def dma_xt(c):
            # issued from the Pool queue: runs concurrently with SP's weight DMAs
            cc = slice(c * CHUNK, (c + 1) * CHUNK)
            t = p_xt.tile([128, 16, CHUNK], dt.bfloat16, tag="xt", name="xtt")
            for d0 in range(0, 16, 4):
                nc.gpsimd.dma_start(t[:, d0:d0 + 4, :], xt[:, d0:d0 + 4, cc])
            return t

        # p-state pre-ramp: spin PE on throwaway matmuls over memset data
        # from t~0 so the tensor clock is at full speed (ramp needs ~3us of
        # continuous execution) when the first real weights arrive
        warm = p_warm.tile([128, CHUNK], dt.bfloat16, tag="warm")
        nc.vector.memset(warm[:], 0.0)
        wps = ps_ms.tile([128, CHUNK], dt.float32, tag="ms")
        for _ in range(8):
            nc.tensor.matmul(wps[:], warm[:, 0:128], warm[:], start=True, stop=True)

        # chunk-0 dependencies first: wk0 (quartered so the first proj
        # matmuls start ASAP), rope tables, wk1, wv, then the rest
        for d0 in range(0, 16, 4):
            nc.sync.dma_start(wk_sb[0][:, d0:d0 + 4, :], wk[0, :, d0:d0 + 4, :])
        xts0 = p_xt.tile([128, 16, CHUNK], dt.bfloat16, tag="xt", name="xtt")
        for d0 in range(0, 16, 2):  # eighths: finer-grained arrival
            nc.gpsimd.dma_start(xts0[:, d0:d0 + 2, :], xt[:, d0:d0 + 2, 0:CHUNK])
        nc.sync.dma_start(rcos_sb[:], rcos[:])
        nc.sync.dma_start(rsin_sb[:], rsin[:])
        nc.sync.dma_start(wk_sb[1][:], wk[1])
        nc.sync.dma_start(wv_sb[:], wv[:])
        for h in range(HL):
            nc.sync.dma_start(wq_sb[h][:], wq[h])
        for m in range(3):
            nc.sync.dma_start(mc_sb[m][:], mconst[m, :, :])
        for h in range(HL):
            nc.sync.dma_start(wo_sb[h][:], wo[h, :, :])

        qt_sb = {}   # (h, c) -> tile, chunk-local
        kt_sb = {(h, c): p_kt.tile([128, CHUNK], dt.bfloat16, tag="kt", name="ktt")
                 for h in range(KL) for c in range(NC_CHUNK)}
        v_sb = [p_v.tile([128, KL * H], dt.bfloat16, tag="v", name="vt") for _ in range(NBLK)]
        enc_sb = {}

        def rope(ps, cc, dst):
            # dst[0:64]  = ps[0:64]*cos - ps[64:128]*sin
            # dst[64:128]= ps[64:128]*cos + ps[0:64]*sin
            # rcos/rsin tables carry the 64-row block duplicated to 128 rows.
            # Muls (PSUM reads) on DVE, partition-aligned; sub/add on Pool.
            tc_ = p_rt.tile([128, CHUNK], dt.float32, tag="rt")
            ts_ = p_rt.tile([128, CHUNK], dt.float32, tag="rt")
            yield nc.vector.tensor_mul(tc_[:], ps[:], rcos_sb[:, cc])
            yield nc.vector.tensor_mul(ts_[0:64, :], ps[64:128, :], rsin_sb[0:64, cc])
            yield nc.vector.tensor_mul(ts_[64:128, :], ps[0:64, :], rsin_sb[64:128, cc])
            yield nc.gpsimd.tensor_sub(dst[0:64, :], tc_[0:64, :], ts_[0:64, :])
            yield nc.gpsimd.tensor_add(dst[64:128, :], tc_[64:128, :], ts_[64:128, :])

        def proj_gen(c, xts):
            """q/k/v projections for chunk c; yields between small slices.
            Order: k heads, q head 0 (so the next chunk's first attention
            head can start promptly), v blocks, then q heads 1-3."""
            cc = slice(c * CHUNK, (c + 1) * CHUNK)

            def kproj(h):
                ps = ps_pj.tile([128, CHUNK], dt.float32, tag="pj")
                for d0 in range(0, 16, 4):
                    for d in range(d0, d0 + 4):
                        nc.tensor.matmul(ps[:], wk_sb[h][:, d, :], xts[:, d, :],
                                         start=(d == 0), stop=(d == 15))
                    yield
                for _ in rope(ps, cc, kt_sb[(h, c)]):
                    yield

            def vproj(p):
                ps = ps_pj.tile([128, KL * H], dt.float32, tag="pj")
                for d0 in range(0, 16, 4):
                    for d in range(d0, d0 + 4):
                        nc.tensor.matmul(ps[:], xts[:, d, p * 128:(p + 1) * 128], wv_sb[:, d, :],
                                         start=(d == 0), stop=(d == 15))
                    yield
                nc.vector.tensor_copy(v_sb[c * 4 + p][:], ps[:])
                yield

            def qproj(h):
                ps = ps_pj.tile([128, CHUNK], dt.float32, tag="pj")
                for d0 in range(0, 16, 4):
                    for d in range(d0, d0 + 4):
                        nc.tensor.matmul(ps[:], wq_sb[h][:, d, :], xts[:, d, :],
                                         start=(d == 0), stop=(d == 15))
                    yield
                qt = p_qt.tile([128, CHUNK], dt.bfloat16, tag="qt", name="qtt")
                for _ in rope(ps, cc, qt):
                    yield
                qt_sb[(h, c)] = qt

            units = [kproj(0), kproj(1)] + [vproj(p) for p in range(4)] \
                + [qproj(h) for h in range(HL)]
            for u in units:
                yield from u

        def oproj_gen(c, lo=0, hi=16):
            """output projection tiles [lo,hi) for chunk c; yields per tile.
            PSUM alternates ps_op/ps_ms (double-buffer); PSUM->SBUF copies go
            to Act in phases where it has slack, else DVE."""
            for i in range(lo, hi):
                p, dc = i // 4, i % 4
                tq = c * 4 + p
                # double-buffer PSUM via ps_ms (free of dn tiles now)
                pool = ps_ms if i % 2 else ps_op
                ps = pool.tile([128, 512], dt.float32,
                               tag="ms" if pool is ps_ms else "op")
                for h in range(HL):
                    nc.tensor.matmul(ps[:], enc_sb[(c, h)][:, p * 128:(p + 1) * 128],
                                     wo_sb[h][:, dc * 512:(dc + 1) * 512],
                                     start=(h == 0), stop=(h == HL - 1))
                orow = out[tq * 128:(tq + 1) * 128, dc * 512:(dc + 1) * 512]
                ot = p_ost.tile([128, 512], dt.bfloat16, tag="ost")
                if c == 3:        # tail: Act is idlest there
                    nc.scalar.copy(ot[:], ps[:])
                else:             # keep Act pure tanh/exp during attention
                    nc.vector.tensor_copy(ot[:], ps[:])
                nc.sync.dma_start(orow, ot[:])
                yield

        def attn_head_gen(c, h):
            kv = h // 2
            blocks = _band(c)
            n = len(blocks)
            qt = qt_sb[(h, c)]
            pv = ps_pv.tile([128, CHUNK], dt.float32, tag="pv")
            es = p_es.tile([128, CHUNK], dt.bfloat16, tag="es")
            lps = {}
            es_tiles = {}

            def emit_qk(i):
                j, off, w, trim = blocks[i]
                lp = ps_lp.tile([128, CHUNK], dt.float32, tag="lp")
                lps[i] = lp
                nc.tensor.matmul(lp[:, 0:w], kt_sb[(kv, j // 4)][:, (j % 4) * 128:(j % 4 + 1) * 128],
                                 qt[:, off:off + w], start=True, stop=(trim is None))
                if trim is not None:
                    # fold the triangle mask in as an additive -1e5 bias:
                    # lp[:, tri cols] += I.T @ mask_bias  (53ns PE pass)
                    kind, col = trim
                    msk = mc_sb[1] if kind == 'u' else mc_sb[2]
                    lc = col - off
                    nc.tensor.matmul(lp[:, lc:lc + 128], mc_sb[0][:], msk[:],
                                     start=False, stop=True, skip_group_check=True)

            def finish(i):
                j, off, w, trim = blocks[i]
                lp = lps.pop(i)
                nc.scalar.activation(lp[:, 0:w], lp[:, 0:w], AF.Tanh, scale=1.0 / SOFT_CAP)
                e = p_e.tile([128, CHUNK], dt.bfloat16, tag="e")
                nc.scalar.activation(e[:, 0:w], lp[:, 0:w], AF.Exp, scale=SOFT_CAP)
                # chunk 3 has no proj filler: its esum rides the idle Pool
                eng = nc.gpsimd if c >= 2 else nc.vector
                if i == 0:
                    assert off == 0 and w == CHUNK
                    eng.tensor_copy(es[:], e[:])
                else:
                    eng.tensor_add(es[:, off:off + w], es[:, off:off + w], e[:, 0:w])
                nc.tensor.matmul(pv[:, off:off + w], v_sb[j][:, kv * 128:(kv + 1) * 128],
                                 e[:, 0:w], start=(i == 0), stop=(i == n - 1),
                                 skip_group_check=True)

            LOOKAHEAD = 2
            for i in range(min(LOOKAHEAD, n)):
                emit_qk(i)
            for i in range(n):
                if i + LOOKAHEAD < n:
                    emit_qk(i + LOOKAHEAD)
                finish(i)
                yield
            # head epilogue: denominator, reciprocal, broadcast, normalize
            # denominator + broadcast in one Pool ucode op (output is the
            # partition-replicated sum), freeing PE of the dn matmul and
            # ps_ms of the dn tile
            dnb = p_rc.tile([128, CHUNK], dt.float32, tag="dnb")
            nc.gpsimd.partition_all_reduce(dnb[:], es[:], 128, bass_isa.ReduceOp.add)
            rcb = p_rc.tile([128, CHUNK], dt.bfloat16, tag="bcs")
            with nc.allow_low_precision(reason="bf16 softmax normalizer is ample"):
                nc.vector.reciprocal(rcb[:], dnb[:])
            enc = p_enc.tile([128, CHUNK], dt.bfloat16, tag="enc")
            nc.vector.tensor_mul(enc[:], pv[:], rcb[:])
            enc_sb[(c, h)] = enc
            yield

        def pump_all(gens):
            for g in gens:
                for _ in g:
                    pass

        # --- schedule: flattened stream of 16 (chunk, head) attention units
        # with proj/oproj generators registered as fillers when their deps
        # are met, paced globally so PE stays interleaved end to end ---
        xts1 = dma_xt(1)
        g0 = proj_gen(0, xts0)
        glen = {}           # gen -> remaining yield estimate

        def mk(g, n):
            glen[g] = n
            return g

        # yields per proj_gen (2k*9 + 4v*5 + 4q*9) + 1 so the trailing
        # StopIteration call still runs the generator's tail code (the last
        # q head's qt_sb registration happens after its final yield)
        PROJ_Y = 75
        projg = {0: mk(g0, PROJ_Y), 1: mk(proj_gen(1, xts1), PROJ_Y)}
        filler = [g0, projg[1]]

        def pump_gen(g):
            try:
                next(g)
                glen[g] -= 1
                return True
            except StopIteration:
                glen[g] = 0
                return False

        fi = 0

        def pump(k):
            nonlocal fi
            misses = 0
            while k > 0 and misses < len(filler):
                g = filler[fi % len(filler)]
                fi += 1
                if glen.get(g, 0) > 0 and pump_gen(g):
                    k -= 1
                    misses = 0
                else:
                    misses += 1

        # prologue: emit proj(0) up to the first q head, queue PE lookahead
        while (0, 0) not in qt_sb:
            pump_gen(g0)
        pump(8)

        Y_total = sum((len(_band(c)) + 1) * HL for c in range(NC_CHUNK))
        y_done = 0
        pump_acc = 0.0
        # oproj registration points, tuned so the act-bound attn(3) phase
        # keeps enough PE filler: oproj(0) splits across attn(1)/(2),
        # oproj(1)+(2) land in attn(3), oproj(3) drains in the tail
        defer = {(1, 0): [(oproj_gen(0, 0, 8), 8)],
                 (2, 0): [(oproj_gen(0, 8, 16), 8)],
                 (3, 0): [(oproj_gen(1), 16), (oproj_gen(2), 16)]}
        for c in range(NC_CHUNK):
            for h in range(HL):
                if h == 0 and c + 1 <= 3 and c + 1 not in projg:
                    projg[c + 1] = mk(proj_gen(c + 1, dma_xt(c + 1)), PROJ_Y)
                    filler.append(projg[c + 1])
                for g, n in defer.get((c, h), []):
                    filler.append(mk(g, n))
                gq = projg.get(c)
                while (h, c) not in qt_sb and glen.get(gq, 0) > 0:
                    pump_gen(gq)
                for _ in attn_head_gen(c, h):
                    y_done += 1
                    left = Y_total - y_done
                    rem = sum(glen.get(g2, 0) for g2 in filler)
                    if left > 0:
                        # proportional pacing via fractional accumulator so
                        # filler never runs dry before the stream ends
                        pump_acc += rem / left
                        n = int(pump_acc)
                        if n:
                            pump_acc -= n
                            pump(n)
                    else:
                        pump(rem)
        filler.append(mk(oproj_gen(3), 16))
        for g in filler:
            while glen.get(g, 0) > 0:
                if not pump_gen(g):
                    break

    nc.compile()
    return nc


def _rope_tables(positions):
    frac = 2.0 * np.arange(64) / H
    timescale = 10000.0 ** frac
    ang = positions[None, :].astype(np.float64) / timescale[:, None]
    cos = np.cos(ang).astype(np.float16)
    sin = np.sin(ang).astype(np.float16)
    # rows duplicated so both halves can be handled with [128,*] ops
    return (np.concatenate([cos, cos], axis=0),
            np.concatenate([sin, sin], axis=0))


def _build_mconst():
    sig = np.arange(128)[:, None]
    tau = np.arange(128)[None, :]
    mc = np.zeros((3, 128, 128), dtype=np.float32)
    mc[0] = (sig == tau)                   # identity (mask-add lhsT)
    mc[1] = np.where(sig <= tau, 0.0, -1e5)  # keep upper incl diag
    mc[2] = np.where(sig > tau, 0.0, -1e5)   # keep strict lower
    return mc.astype(bf16)


def _build_in_maps(x, segment_pos, attn_mask, wq, wkv, wo):
    x = np.asarray(x)
    segment_pos = np.asarray(segment_pos)
    wq = np.asarray(wq)
    wkv = np.asarray(wkv)
    wo = np.asarray(wo)
    mc_np = _build_mconst()
    scale = H ** -0.5

    def dslice(a):
        # [D, C] -> [128, 16, C]: partition-major over 128-row D slices
        return np.ascontiguousarray(
            a.reshape(16, 128, a.shape[1]).transpose(1, 0, 2))

    in_maps = []
    for c in range(8):
        b, r = c // 4, c % 4
        cos, sin = _rope_tables(segment_pos[b])
        in_maps.append({
            "xt": dslice(x[b].T).astype(bf16),
            "wq": np.stack([dslice(wq[4 * r + h] * scale) for h in range(4)]).astype(bf16),
            "wk": np.stack([dslice(wkv[0, 2 * r + h]) for h in range(2)]).astype(bf16),
            "wv": dslice(np.concatenate(
                [wkv[1, 2 * r], wkv[1, 2 * r + 1]], axis=1)).astype(bf16),
            "wo": wo[4 * r:4 * r + 4].astype(bf16),
            "rcos": cos, "rsin": sin,
            "mconst": mc_np,
        })
    return in_maps


def kernel(x, segment_pos, attn_mask, wq, wkv, wo):
    global LAST_RESULT
    from concourse.bass_utils import run_bass_kernel_spmd

    nc = _build_graph()
    in_maps = _build_in_maps(x, segment_pos, attn_mask, wq, wkv, wo)

    res = run_bass_kernel_spmd(nc, in_maps, core_ids=list(range(8)))
    LAST_RESULT = res
    out = np.zeros((B, T, D), dtype=np.float32)
    for c in range(8):
        out[c // 4] += res.results[c]["out"].astype(np.float32)
    return out
